# revision 11
# baseline (speedup 1.0000x reference)
"""MetapathAttentionLayer Trainium2 kernel (v2: packed node-metapath layout).

Math (per node n):
    scores[n, m] = sum_d x[m, n, d] * W[d, m]
    att = softmax(relu(scores), axis=m)      (8 metapaths)
    out[n, :] = elu(sum_m att[n, m] * x[m, n, :])

Strategy: shard nodes across 8 cores (data parallel).  Per core, nodes are
packed so SBUF partition p = (node%32)*4 + metapath' holds one (node,
metapath) row of x, split into two halves (metapaths 0-3 / 4-7).  Per
region of 1024 nodes (32 tiles of 32 nodes):
  - scores: DVE tensor_tensor multiply against a replicated-W pattern
    (per-partition W column), then a batched binary-tree reduction over d
    (all tree levels are single DVE ops covering every tile).
  - softmax over m: ACT relu+exp; sum over the 8 metapaths of each node
    via PE matmul with a constant block-indicator stationary; DVE
    reciprocal; broadcast back with a second const matmul; weights applied
    on GPSIMD.
  - pooling: GPSIMD local_scatter packs attention weights into 32-wide
    stationaries (4 diagonals each); PE matmuls contract the (node,
    metapath) partition dim, accumulating both halves into PSUM.
  - elu(x) = relu(x) + exp(-relu(-x)) - 1: ACT x3 + GPSIMD combine,
    bf16 output DMA.
"""

from contextlib import ExitStack

import numpy as np
import ml_dtypes

import concourse.bass as bass
import concourse.tile as tile
from concourse import bacc, mybir, library_config
import concourse.bass_utils as bass_utils

F32 = mybir.dt.float32
BF16 = mybir.dt.bfloat16
I16 = mybir.dt.int16
ALU = mybir.AluOpType
ACTF = mybir.ActivationFunctionType

NMETA = 8
N = 100000
D = 128
NCORES = 8
NC_RAW = N // NCORES          # 12500 nodes per core
NC_PAD = 12800                # 400 tiles of 32 nodes
NTILES = NC_PAD // 32         # 400
RTILE = 32                    # tiles per region (1024 nodes)


def _region_sizes():
    """Tiles per region: small ramp-in/out regions to shorten pipeline
    fill and drain, full 32-tile regions in the middle."""
    head = [16, 16]
    tail = [16]
    mid_tiles = NTILES - sum(head) - sum(tail)
    assert mid_tiles % RTILE == 0
    return head + [RTILE] * (mid_tiles // RTILE) + tail


def kernel_body(tc, out_d, xa_d, xb_d, wba_d, wbb_d, blk4_d, blk4t_d, sidx_d):
    nc = tc.nc
    sizes = _region_sizes()
    starts = [sum(sizes[:i]) for i in range(len(sizes))]
    R = len(sizes)
    with ExitStack() as ctx:
        const = ctx.enter_context(tc.tile_pool(name="const", bufs=1))
        xpool = ctx.enter_context(tc.tile_pool(name="x", bufs=4))
        ppool = ctx.enter_context(tc.tile_pool(name="prod", bufs=2))
        tpool = ctx.enter_context(tc.tile_pool(name="tree", bufs=3))
        spool = ctx.enter_context(tc.tile_pool(name="smalls", bufs=3))
        scat = ctx.enter_context(tc.tile_pool(name="scat", bufs=3))
        epool = ctx.enter_context(tc.tile_pool(name="elu", bufs=2))
        opool = ctx.enter_context(tc.tile_pool(name="osb", bufs=3))
        psum = ctx.enter_context(tc.tile_pool(name="ps", bufs=3, space="PSUM"))
        psum_s = ctx.enter_context(tc.tile_pool(name="pss", bufs=2, space="PSUM"))

        wba = const.tile([128, D], BF16)
        nc.sync.dma_start(wba[:], wba_d[:])
        wbb = const.tile([128, D], BF16)
        nc.sync.dma_start(wbb[:], wbb_d[:])
        blk4 = const.tile([128, 32], BF16)
        nc.sync.dma_start(blk4[:], blk4_d[:])
        blk4t = const.tile([32, 128], F32)
        nc.sync.dma_start(blk4t[:], blk4t_d[:])
        sidx = const.tile([128, RTILE], I16)
        nc.sync.dma_start(sidx[:], sidx_d[:])
        nc.gpsimd.load_library(library_config.local_scatter)

        st = {}   # region -> dict of live tiles

        def stage_dma(r):
            nt = sizes[r]
            fw = nt * D
            d = {"nt": nt}
            for h, x_d in (("a", xa_d), ("b", xb_d)):
                xt = xpool.tile([128, RTILE * D], BF16, tag=f"X{h}",
                                name=f"X{h}")
                nc.sync.dma_start(
                    xt[:, :fw],
                    x_d[:, starts[r]:starts[r] + nt, :].rearrange(
                        "p t d -> p (t d)"))
                d[f"X{h}"] = xt
            st[r] = d

        def stage_scores(r):
            """mult + tree + relu/exp + Σe matmuls (no recip/att yet)."""
            d = st[r]
            nt = d["nt"]
            fw = nt * D
            s = spool.tile([128, 2 * RTILE], F32, tag="s")
            for hi, (h, wb) in enumerate((("a", wba), ("b", wbb))):
                P = ppool.tile([128, RTILE * D], BF16, tag=f"P{h}",
                               name=f"P{h}")
                nc.vector.tensor_tensor(
                    out=P[:, :fw].rearrange("p (t d) -> p t d", t=nt),
                    in0=d[f"X{h}"][:, :fw].rearrange("p (t d) -> p t d", t=nt),
                    in1=wb[:].unsqueeze(1).broadcast_to([128, nt, D]),
                    op=ALU.mult,
                )
                cur = P
                w = D // 2
                while w >= 2:
                    nxt = tpool.tile([128, RTILE * w], BF16, tag=f"T{h}{w}",
                                     name=f"T{h}{w}")
                    cv = cur[:, :nt * 2 * w].rearrange(
                        "p (t d) -> p t d", t=nt)
                    nc.vector.tensor_tensor(
                        out=nxt[:, :nt * w].rearrange(
                            "p (t d) -> p t d", t=nt),
                        in0=cv[:, :, 0:w],
                        in1=cv[:, :, w:2 * w],
                        op=ALU.add,
                    )
                    cur = nxt
                    w //= 2
                cv = cur[:, :nt * 2].rearrange("p (t d) -> p t d", t=nt)
                nc.vector.tensor_tensor(
                    out=s[:, hi * nt:hi * nt + nt].unsqueeze(2),
                    in0=cv[:, :, 0:1],
                    in1=cv[:, :, 1:2],
                    op=ALU.add,
                )
            sr = spool.tile([128, 2 * RTILE], BF16, tag="sr")
            nc.scalar.activation(sr[:, :2 * nt], s[:, :2 * nt], ACTF.Relu)
            e = spool.tile([128, 2 * RTILE], BF16, tag="e")
            nc.scalar.activation(e[:, :2 * nt], sr[:, :2 * nt], ACTF.Exp)
            # sums into [0:32, 0:nt] of the shared small psum tile
            sm = psum_s.tile([128, 2 * RTILE], F32, tag="sm")
            nc.tensor.matmul(out=sm[0:32, 0:nt], lhsT=blk4[:],
                             rhs=e[:, 0:nt], start=True, stop=False)
            nc.tensor.matmul(out=sm[0:32, 0:nt], lhsT=blk4[:],
                             rhs=e[:, nt:2 * nt], start=False, stop=True)
            d["e"] = e
            d["sm"] = sm

        def stage_att(r):
            """recip + inv broadcast + att + scatter."""
            d = st[r]
            nt = d["nt"]
            e, sm = d["e"], d["sm"]
            inv = spool.tile([32, RTILE], F32, tag="inv")
            nc.vector.reciprocal(inv[:, :nt], sm[0:32, 0:nt])
            nc.tensor.matmul(out=sm[:, RTILE:RTILE + nt], lhsT=blk4t[:],
                             rhs=inv[:, :nt], start=True, stop=True)
            att = spool.tile([128, 2 * RTILE], BF16, tag="att")
            nc.vector.scalar_tensor_tensor(
                out=att[:, :2 * nt].rearrange("p (h t) -> p h t", h=2),
                in0=e[:, :2 * nt].rearrange("p (h t) -> p h t", h=2),
                scalar=1.0,
                in1=sm[:, RTILE:RTILE + nt].unsqueeze(1).broadcast_to(
                    [128, 2, nt]),
                op0=ALU.mult, op1=ALU.mult,
            )
            for hi, h in enumerate(("a", "b")):
                S = scat.tile([128, RTILE * 32], BF16, tag=f"S{h}",
                              name=f"S{h}")
                nc.gpsimd.local_scatter(
                    S[:, :nt * 32], att[:, hi * nt:hi * nt + nt],
                    sidx[:, :nt], channels=128,
                    num_elems=nt * 32, num_idxs=nt)
                d[f"S{h}"] = S

        def stage_pool(r):
            d = st[r]
            nt = d["nt"]
            pool_ps = psum.tile([128, RTILE * 32], F32, tag="pool")
            for tt in range(nt):
                po = 32 * (tt & 3)
                co = D * (tt >> 2)
                nc.tensor.matmul(
                    out=pool_ps[po:po + 32, co:co + D],
                    lhsT=d["Sa"][:, 32 * tt:32 * tt + 32],
                    rhs=d["Xa"][:, D * tt:D * tt + D],
                    start=True, stop=False, tile_position=(0, po))
                nc.tensor.matmul(
                    out=pool_ps[po:po + 32, co:co + D],
                    lhsT=d["Sb"][:, 32 * tt:32 * tt + 32],
                    rhs=d["Xb"][:, D * tt:D * tt + D],
                    start=False, stop=True, tile_position=(0, po))
            d["pool"] = pool_ps

        def stage_elu(r):
            d = st[r]
            nt = d["nt"]
            nn = nt * 32
            pool_ps = d["pool"]
            rl = epool.tile([128, RTILE * 32], BF16, tag="rl")
            nc.scalar.activation(rl[:, :nn], pool_ps[:, :nn], ACTF.Relu)
            t2 = epool.tile([128, RTILE * 32], BF16, tag="t2")
            nc.scalar.activation(t2[:, :nn], pool_ps[:, :nn], ACTF.Relu,
                                 scale=-1.0)
            e2 = epool.tile([128, RTILE * 32], BF16, tag="e2")
            nc.scalar.activation(e2[:, :nn], t2[:, :nn], ACTF.Exp,
                                 scale=-1.0)
            cmb = epool.tile([128, RTILE * 32], BF16, tag="cmb")
            nc.vector.tensor_tensor(
                out=cmb[:, :nn], in0=e2[:, :nn], in1=rl[:, :nn], op=ALU.add)
            out_sb = opool.tile([128, RTILE * 32], BF16, tag="osb")
            nc.vector.tensor_scalar(
                out_sb[:, :nn], cmb[:, :nn], -1.0, None, ALU.add)
            nc.sync.dma_start(
                out_d[:, starts[r] * 32:starts[r] * 32 + nn], out_sb[:, :nn])
            del st[r]

        # software pipeline, stages offset so every stage's inputs were
        # produced at least one iteration earlier:
        #   iter k: att(k-2) | elu(k-3) | dma(k) | scores(k-1) | pool(k-2)
        for k in range(R + 3):
            if 2 <= k <= R + 1:
                stage_att(k - 2)
            if 3 <= k:
                stage_elu(k - 3)
            if k < R:
                stage_dma(k)
            if 1 <= k <= R:
                stage_scores(k - 1)
            if 2 <= k <= R + 1:
                stage_pool(k - 2)


def host_inputs(x_np, w_np):
    """Build per-core input maps from full fp32 inputs."""
    q = np.arange(128) >> 2          # node-in-tile per partition
    mi = np.arange(128) & 3          # metapath-within-half per partition

    wba = np.ascontiguousarray(w_np.T[mi, :]).astype(ml_dtypes.bfloat16)
    wbb = np.ascontiguousarray(w_np.T[4 + mi, :]).astype(ml_dtypes.bfloat16)
    blk4 = (np.arange(32)[None, :] == q[:, None]).astype(ml_dtypes.bfloat16)
    blk4t = np.ascontiguousarray(blk4.T).astype(np.float32)
    sidx = (32 * np.arange(RTILE)[None, :] + q[:, None]).astype(np.int16)

    in_maps = []
    for c in range(NCORES):
        xs = x_np[:, c * NC_RAW:(c + 1) * NC_RAW, :]
        xp = np.zeros((NMETA, NC_PAD, D), dtype=ml_dtypes.bfloat16)
        xp[:, :NC_RAW, :] = xs.astype(ml_dtypes.bfloat16)
        arr = xp.reshape(NMETA, NTILES, 32, D)
        # partition p = q*4 + mi  ->  [q, mi, t, d]
        xa = np.ascontiguousarray(
            arr[0:4].transpose(2, 0, 1, 3).reshape(128, NTILES, D))
        xb = np.ascontiguousarray(
            arr[4:8].transpose(2, 0, 1, 3).reshape(128, NTILES, D))
        in_maps.append({"xa": xa, "xb": xb, "wba": wba, "wbb": wbb,
                        "blk4": blk4, "blk4t": blk4t, "sidx": sidx})
    return in_maps


def unshard(res):
    """Per-core [128, NC_PAD] bf16 psum-slot layout -> full [N, D] f32."""
    full = np.empty((NCORES, NC_RAW, D), dtype=np.float32)
    sizes = _region_sizes()
    for c in range(NCORES):
        o = np.asarray(res.results[c]["out"]).astype(np.float32)
        parts = []
        col = 0
        for nt in sizes:
            nn = nt * 32
            b = o[:, col:col + nn].reshape(4, 32, nt // 4, D)
            # node-in-region = 32*(cblk*4 + pblk) + q
            parts.append(b.transpose(2, 0, 1, 3).reshape(nn, D))
            col += nn
        full[c] = np.concatenate(parts, axis=0)[:NC_RAW]
    return full.reshape(N, D)


_CACHE = {}


def build():
    if "nc" in _CACHE:
        return _CACHE["nc"]
    nc = bacc.Bacc("TRN2", target_bir_lowering=False, debug=False,
                   num_devices=NCORES)
    xa = nc.dram_tensor("xa", [128, NTILES, D], BF16, kind="ExternalInput").ap()
    xb = nc.dram_tensor("xb", [128, NTILES, D], BF16, kind="ExternalInput").ap()
    wba = nc.dram_tensor("wba", [128, D], BF16, kind="ExternalInput").ap()
    wbb = nc.dram_tensor("wbb", [128, D], BF16, kind="ExternalInput").ap()
    blk4 = nc.dram_tensor("blk4", [128, 32], BF16, kind="ExternalInput").ap()
    blk4t = nc.dram_tensor("blk4t", [32, 128], F32, kind="ExternalInput").ap()
    sidx = nc.dram_tensor("sidx", [128, RTILE], I16, kind="ExternalInput").ap()
    out = nc.dram_tensor("out", [128, NC_PAD], BF16, kind="ExternalOutput").ap()
    with tile.TileContext(nc) as tc:
        kernel_body(tc, out, xa, xb, wba, wbb, blk4, blk4t, sidx)
    nc.compile()
    _CACHE["nc"] = nc
    return nc


def run(input, W, trace=False, **trace_kwargs):
    x_np = np.asarray(input, dtype=np.float32)
    w_np = np.asarray(W, dtype=np.float32)
    nc = build()
    in_maps = host_inputs(x_np, w_np)
    res = bass_utils.run_bass_kernel_spmd(
        nc, in_maps, core_ids=list(range(NCORES)), trace=trace, **trace_kwargs)
    return unshard(res), res


def kernel(input, W):
    out, _ = run(input, W, trace=False)
    return out


# revision 12
# speedup vs baseline: 1.0165x; 1.0165x over previous
"""MetapathAttentionLayer Trainium2 kernel (v2: packed node-metapath layout).

Math (per node n):
    scores[n, m] = sum_d x[m, n, d] * W[d, m]
    att = softmax(relu(scores), axis=m)      (8 metapaths)
    out[n, :] = elu(sum_m att[n, m] * x[m, n, :])

Strategy: shard nodes across 8 cores (data parallel).  Per core, nodes are
packed so SBUF partition p = (node%32)*4 + metapath' holds one (node,
metapath) row of x, split into two halves (metapaths 0-3 / 4-7).  Per
region of 1024 nodes (32 tiles of 32 nodes):
  - scores: DVE tensor_tensor multiply against a replicated-W pattern
    (per-partition W column), then a batched binary-tree reduction over d
    (all tree levels are single DVE ops covering every tile).
  - softmax over m: ACT relu+exp; sum over the 8 metapaths of each node
    via PE matmul with a constant block-indicator stationary; DVE
    reciprocal; broadcast back with a second const matmul; weights applied
    on GPSIMD.
  - pooling: GPSIMD local_scatter packs attention weights into 32-wide
    stationaries (4 diagonals each); PE matmuls contract the (node,
    metapath) partition dim, accumulating both halves into PSUM.
  - elu(x) = relu(x) + exp(-relu(-x)) - 1: ACT x3 + GPSIMD combine,
    bf16 output DMA.
"""

from contextlib import ExitStack

import numpy as np
import ml_dtypes

import concourse.bass as bass
import concourse.tile as tile
from concourse import bacc, mybir, library_config
import concourse.bass_utils as bass_utils

F32 = mybir.dt.float32
BF16 = mybir.dt.bfloat16
I16 = mybir.dt.int16
ALU = mybir.AluOpType
ACTF = mybir.ActivationFunctionType

NMETA = 8
N = 100000
D = 128
NCORES = 8
NC_RAW = N // NCORES          # 12500 nodes per core
NC_PAD = 12800                # 400 tiles of 32 nodes
NTILES = NC_PAD // 32         # 400
RTILE = 32                    # tiles per region (1024 nodes)


def _region_sizes():
    """Tiles per region: small ramp-in/out regions to shorten pipeline
    fill and drain, full 32-tile regions in the middle."""
    head = [16, 16]
    tail = [16]
    mid_tiles = NTILES - sum(head) - sum(tail)
    assert mid_tiles % RTILE == 0
    return head + [RTILE] * (mid_tiles // RTILE) + tail


def kernel_body(tc, out_d, xa_d, xb_d, wba_d, wbb_d, blk4_d, blk4t_d, sidx_d):
    nc = tc.nc
    sizes = _region_sizes()
    starts = [sum(sizes[:i]) for i in range(len(sizes))]
    R = len(sizes)
    with ExitStack() as ctx:
        const = ctx.enter_context(tc.tile_pool(name="const", bufs=1))
        xpool = ctx.enter_context(tc.tile_pool(name="x", bufs=4))
        ppool = ctx.enter_context(tc.tile_pool(name="prod", bufs=2))
        tpool = ctx.enter_context(tc.tile_pool(name="tree", bufs=3))
        spool = ctx.enter_context(tc.tile_pool(name="smalls", bufs=3))
        scat = ctx.enter_context(tc.tile_pool(name="scat", bufs=3))
        epool = ctx.enter_context(tc.tile_pool(name="elu", bufs=2))
        opool = ctx.enter_context(tc.tile_pool(name="osb", bufs=3))
        psum = ctx.enter_context(tc.tile_pool(name="ps", bufs=3, space="PSUM"))
        psum_s = ctx.enter_context(tc.tile_pool(name="pss", bufs=2, space="PSUM"))

        wba = const.tile([128, D], BF16)
        nc.sync.dma_start(wba[:], wba_d[:])
        wbb = const.tile([128, D], BF16)
        nc.sync.dma_start(wbb[:], wbb_d[:])
        blk4 = const.tile([128, 32], BF16)
        nc.sync.dma_start(blk4[:], blk4_d[:])
        blk4t = const.tile([32, 128], F32)
        nc.sync.dma_start(blk4t[:], blk4t_d[:])
        sidx = const.tile([128, RTILE], I16)
        nc.sync.dma_start(sidx[:], sidx_d[:])
        nc.gpsimd.load_library(library_config.local_scatter)

        st = {}   # region -> dict of live tiles

        def stage_dma(r):
            nt = sizes[r]
            fw = nt * D
            d = {"nt": nt}
            for h, x_d in (("a", xa_d), ("b", xb_d)):
                xt = xpool.tile([128, RTILE * D], BF16, tag=f"X{h}",
                                name=f"X{h}")
                nc.sync.dma_start(
                    xt[:, :fw],
                    x_d[:, starts[r]:starts[r] + nt, :].rearrange(
                        "p t d -> p (t d)"))
                d[f"X{h}"] = xt
            st[r] = d

        def stage_scores(r):
            """mult + tree + relu/exp + Σe matmuls (no recip/att yet)."""
            d = st[r]
            nt = d["nt"]
            fw = nt * D
            s = spool.tile([128, 2 * RTILE], F32, tag="s")
            for hi, (h, wb) in enumerate((("a", wba), ("b", wbb))):
                P = ppool.tile([128, RTILE * D], BF16, tag=f"P{h}",
                               name=f"P{h}")
                nc.vector.tensor_tensor(
                    out=P[:, :fw].rearrange("p (t d) -> p t d", t=nt),
                    in0=d[f"X{h}"][:, :fw].rearrange("p (t d) -> p t d", t=nt),
                    in1=wb[:].unsqueeze(1).broadcast_to([128, nt, D]),
                    op=ALU.mult,
                )
                cur = P
                w = D // 2
                while w >= 2:
                    nxt = tpool.tile([128, RTILE * w], BF16, tag=f"T{h}{w}",
                                     name=f"T{h}{w}")
                    cv = cur[:, :nt * 2 * w].rearrange(
                        "p (t d) -> p t d", t=nt)
                    nc.vector.tensor_tensor(
                        out=nxt[:, :nt * w].rearrange(
                            "p (t d) -> p t d", t=nt),
                        in0=cv[:, :, 0:w],
                        in1=cv[:, :, w:2 * w],
                        op=ALU.add,
                    )
                    cur = nxt
                    w //= 2
                cv = cur[:, :nt * 2].rearrange("p (t d) -> p t d", t=nt)
                nc.vector.tensor_tensor(
                    out=s[:, hi * nt:hi * nt + nt].unsqueeze(2),
                    in0=cv[:, :, 0:1],
                    in1=cv[:, :, 1:2],
                    op=ALU.add,
                )
            sr = spool.tile([128, 2 * RTILE], BF16, tag="sr")
            nc.scalar.activation(sr[:, :2 * nt], s[:, :2 * nt], ACTF.Relu)
            e = spool.tile([128, 2 * RTILE], BF16, tag="e")
            nc.scalar.activation(e[:, :2 * nt], sr[:, :2 * nt], ACTF.Exp)
            # sums into [0:32, 0:nt] of the shared small psum tile
            sm = psum_s.tile([128, 2 * RTILE], F32, tag="sm")
            nc.tensor.matmul(out=sm[0:32, 0:nt], lhsT=blk4[:],
                             rhs=e[:, 0:nt], start=True, stop=False)
            nc.tensor.matmul(out=sm[0:32, 0:nt], lhsT=blk4[:],
                             rhs=e[:, nt:2 * nt], start=False, stop=True)
            d["e"] = e
            d["sm"] = sm

        def stage_att(r):
            """recip + inv broadcast + att + scatter."""
            d = st[r]
            nt = d["nt"]
            e, sm = d["e"], d["sm"]
            inv = spool.tile([32, RTILE], F32, tag="inv")
            nc.vector.reciprocal(inv[:, :nt], sm[0:32, 0:nt])
            nc.tensor.matmul(out=sm[:, RTILE:RTILE + nt], lhsT=blk4t[:],
                             rhs=inv[:, :nt], start=True, stop=True)
            att = spool.tile([128, 2 * RTILE], BF16, tag="att")
            nc.vector.scalar_tensor_tensor(
                out=att[:, :2 * nt].rearrange("p (h t) -> p h t", h=2),
                in0=e[:, :2 * nt].rearrange("p (h t) -> p h t", h=2),
                scalar=1.0,
                in1=sm[:, RTILE:RTILE + nt].unsqueeze(1).broadcast_to(
                    [128, 2, nt]),
                op0=ALU.mult, op1=ALU.mult,
            )
            for hi, h in enumerate(("a", "b")):
                S = scat.tile([128, RTILE * 32], BF16, tag=f"S{h}",
                              name=f"S{h}")
                nc.gpsimd.local_scatter(
                    S[:, :nt * 32], att[:, hi * nt:hi * nt + nt],
                    sidx[:, :nt], channels=128,
                    num_elems=nt * 32, num_idxs=nt)
                d[f"S{h}"] = S

        def stage_pool(r):
            d = st[r]
            nt = d["nt"]
            pool_ps = psum.tile([128, RTILE * 32], F32, tag="pool")
            for tt in range(nt):
                po = 32 * (tt & 3)
                co = D * (tt >> 2)
                nc.tensor.matmul(
                    out=pool_ps[po:po + 32, co:co + D],
                    lhsT=d["Sa"][:, 32 * tt:32 * tt + 32],
                    rhs=d["Xa"][:, D * tt:D * tt + D],
                    start=True, stop=False, tile_position=(0, po))
                nc.tensor.matmul(
                    out=pool_ps[po:po + 32, co:co + D],
                    lhsT=d["Sb"][:, 32 * tt:32 * tt + 32],
                    rhs=d["Xb"][:, D * tt:D * tt + D],
                    start=False, stop=True, tile_position=(0, po))
            d["pool"] = pool_ps

        def stage_elu_act(r):
            d = st[r]
            nt = d["nt"]
            nn = nt * 32
            pool_ps = d["pool"]
            rl = epool.tile([128, RTILE * 32], BF16, tag="rl")
            nc.scalar.activation(rl[:, :nn], pool_ps[:, :nn], ACTF.Relu)
            t2 = epool.tile([128, RTILE * 32], BF16, tag="t2")
            nc.scalar.activation(t2[:, :nn], pool_ps[:, :nn], ACTF.Relu,
                                 scale=-1.0)
            e2 = epool.tile([128, RTILE * 32], BF16, tag="e2")
            nc.scalar.activation(e2[:, :nn], t2[:, :nn], ACTF.Exp,
                                 scale=-1.0)
            d["rl"], d["e2"] = rl, e2

        def stage_elu_dve(r):
            d = st[r]
            nt = d["nt"]
            nn = nt * 32
            cmb = epool.tile([128, RTILE * 32], BF16, tag="cmb")
            nc.vector.tensor_tensor(
                out=cmb[:, :nn], in0=d["e2"][:, :nn], in1=d["rl"][:, :nn],
                op=ALU.add)
            out_sb = opool.tile([128, RTILE * 32], BF16, tag="osb")
            nc.vector.tensor_scalar(
                out_sb[:, :nn], cmb[:, :nn], -1.0, None, ALU.add)
            nc.sync.dma_start(
                out_d[:, starts[r] * 32:starts[r] * 32 + nn], out_sb[:, :nn])
            del st[r]

        # software pipeline, stages offset so every stage's inputs were
        # produced at least one iteration earlier:
        for k in range(R + 3):
            if 3 <= k:
                stage_elu_act(k - 3)
            if 2 <= k <= R + 1:
                stage_att(k - 2)
            if k < R:
                stage_dma(k)
            if 1 <= k <= R:
                stage_scores(k - 1)
            if 2 <= k <= R + 1:
                stage_pool(k - 2)
            if 3 <= k:
                stage_elu_dve(k - 3)


def host_inputs(x_np, w_np):
    """Build per-core input maps from full fp32 inputs."""
    q = np.arange(128) >> 2          # node-in-tile per partition
    mi = np.arange(128) & 3          # metapath-within-half per partition

    wba = np.ascontiguousarray(w_np.T[mi, :]).astype(ml_dtypes.bfloat16)
    wbb = np.ascontiguousarray(w_np.T[4 + mi, :]).astype(ml_dtypes.bfloat16)
    blk4 = (np.arange(32)[None, :] == q[:, None]).astype(ml_dtypes.bfloat16)
    blk4t = np.ascontiguousarray(blk4.T).astype(np.float32)
    sidx = (32 * np.arange(RTILE)[None, :] + q[:, None]).astype(np.int16)

    in_maps = []
    for c in range(NCORES):
        xs = x_np[:, c * NC_RAW:(c + 1) * NC_RAW, :]
        xp = np.zeros((NMETA, NC_PAD, D), dtype=ml_dtypes.bfloat16)
        xp[:, :NC_RAW, :] = xs.astype(ml_dtypes.bfloat16)
        arr = xp.reshape(NMETA, NTILES, 32, D)
        # partition p = q*4 + mi  ->  [q, mi, t, d]
        xa = np.ascontiguousarray(
            arr[0:4].transpose(2, 0, 1, 3).reshape(128, NTILES, D))
        xb = np.ascontiguousarray(
            arr[4:8].transpose(2, 0, 1, 3).reshape(128, NTILES, D))
        in_maps.append({"xa": xa, "xb": xb, "wba": wba, "wbb": wbb,
                        "blk4": blk4, "blk4t": blk4t, "sidx": sidx})
    return in_maps


def unshard(res):
    """Per-core [128, NC_PAD] bf16 psum-slot layout -> full [N, D] f32."""
    full = np.empty((NCORES, NC_RAW, D), dtype=np.float32)
    sizes = _region_sizes()
    for c in range(NCORES):
        o = np.asarray(res.results[c]["out"]).astype(np.float32)
        parts = []
        col = 0
        for nt in sizes:
            nn = nt * 32
            b = o[:, col:col + nn].reshape(4, 32, nt // 4, D)
            # node-in-region = 32*(cblk*4 + pblk) + q
            parts.append(b.transpose(2, 0, 1, 3).reshape(nn, D))
            col += nn
        full[c] = np.concatenate(parts, axis=0)[:NC_RAW]
    return full.reshape(N, D)


_CACHE = {}


def build():
    if "nc" in _CACHE:
        return _CACHE["nc"]
    nc = bacc.Bacc("TRN2", target_bir_lowering=False, debug=False,
                   num_devices=NCORES)
    xa = nc.dram_tensor("xa", [128, NTILES, D], BF16, kind="ExternalInput").ap()
    xb = nc.dram_tensor("xb", [128, NTILES, D], BF16, kind="ExternalInput").ap()
    wba = nc.dram_tensor("wba", [128, D], BF16, kind="ExternalInput").ap()
    wbb = nc.dram_tensor("wbb", [128, D], BF16, kind="ExternalInput").ap()
    blk4 = nc.dram_tensor("blk4", [128, 32], BF16, kind="ExternalInput").ap()
    blk4t = nc.dram_tensor("blk4t", [32, 128], F32, kind="ExternalInput").ap()
    sidx = nc.dram_tensor("sidx", [128, RTILE], I16, kind="ExternalInput").ap()
    out = nc.dram_tensor("out", [128, NC_PAD], BF16, kind="ExternalOutput").ap()
    with tile.TileContext(nc) as tc:
        kernel_body(tc, out, xa, xb, wba, wbb, blk4, blk4t, sidx)
    nc.compile()
    _CACHE["nc"] = nc
    return nc


def run(input, W, trace=False, **trace_kwargs):
    x_np = np.asarray(input, dtype=np.float32)
    w_np = np.asarray(W, dtype=np.float32)
    nc = build()
    in_maps = host_inputs(x_np, w_np)
    res = bass_utils.run_bass_kernel_spmd(
        nc, in_maps, core_ids=list(range(NCORES)), trace=trace, **trace_kwargs)
    return unshard(res), res


def kernel(input, W):
    out, _ = run(input, W, trace=False)
    return out
